# revision 8
# baseline (speedup 1.0000x reference)
"""GQA kernel for Trainium2, 8 NeuronCores.

Problem: nn_GroupQueryAttention — B=4, S=2048, E=2048, 16 heads / 4 groups,
d_head=128.  out = softmax((x@Wq) (x@Wk)^T / sqrt(d)) (x@Wv) @ Wo + biases.

Sharding: core c -> (batch b = c//2, half = c%2).  Each core handles one
batch and 2 of the 4 KV groups (= 8 of the 16 heads): Wq columns / Wo rows
split by head, Wk/Wv columns split by group.  Each core produces a partial
output projection for its batch; the host sums the two halves.

v4: v3 (C-phase filler pumping) + idle killers found in the v3 trace:
  - one rotating x pool (tags, fine-grained slot deps) instead of scoped
    pools whose closes serialized the DMA stream behind compute;
  - bq's 1024-descriptor gather moved to the idle gpsimd queue so wk/wq
    transfers start immediately; sync-queue transfer order = need order:
    wk, xk, (wq[e], xq0[e]) interleaved, wv, xq1, xv, wo, bo, outs;
  - PSUM-draining DVE ops (kT copies, qh bias adds, vh copies) interleaved
    with their producer matmuls so pool transitions don't stall the PE;
  - C units emit the NEXT unit's first scores tile before their epilogue
    (the scalar engine then flows exp->exp across unit boundaries; v3 lost
    2.4us/unit there), and ps_ctx is drained by an unnormalized bf16 copy
    (cxu) with row-sum/reciprocal/normalize trailing off the critical path.
  - numerics identical to v2/v3 (bf16 matmuls, fp32 accum, bk dropped,
    bv+bo folded host-side, bq folded on-chip, exp without max-subtract).
"""

import sys

sys.path.insert(0, "/opt/trn_rl_repo")

import numpy as np
import ml_dtypes

BF16 = ml_dtypes.bfloat16

B, S, E = 4, 2048, 2048
D = 128            # head dim
HPC = 8            # heads per core
GPC = 2            # groups per core
QC = HPC * D       # 1024 Wq cols per core
KV = GPC * D       # 256 Wk/Wv cols per core
NE = E // D        # 16 contraction chunks
NT = S // D        # 16 t-chunks of 128
QH = 2             # q halves of 1024
QW = S // QH       # 1024
N_CORES = 8

_PROGRAM = None


def _build():
    from contextlib import ExitStack

    import concourse.bass as bass
    import concourse.mybir as mybir
    import concourse.tile as tile
    from concourse import bacc

    F32 = mybir.dt.float32
    BF = mybir.dt.bfloat16
    Exp = mybir.ActivationFunctionType.Exp
    SCALE = 1.0 / float(np.sqrt(D))

    nc = bacc.Bacc("TRN2", target_bir_lowering=False, debug=False)
    xq = nc.dram_tensor("xq", [E, S], BF, kind="ExternalInput")
    xk = nc.dram_tensor("xk", [E, S], BF, kind="ExternalInput")
    xv = nc.dram_tensor("xv", [E, S], BF, kind="ExternalInput")
    wq = nc.dram_tensor("wq", [E, QC], BF, kind="ExternalInput")
    wk = nc.dram_tensor("wk", [E, KV], BF, kind="ExternalInput")
    wv = nc.dram_tensor("wv", [E, KV], BF, kind="ExternalInput")
    wo = nc.dram_tensor("wo", [QC, E], BF, kind="ExternalInput")
    bq = nc.dram_tensor("bq", [QC], F32, kind="ExternalInput")
    bo = nc.dram_tensor("bo", [E], F32, kind="ExternalInput")
    out = nc.dram_tensor("out_p", [S, E], F32, kind="ExternalOutput")

    wq_r = wq.ap().rearrange("(n p) c -> p n c", p=D)   # [128,16,1024]
    wo_r = wo.ap().rearrange("(h p) e -> p h e", p=D)   # [128,8,2048]

    def bcast(dram, n):
        return bass.AP(tensor=dram.ap().tensor, offset=0, ap=[[0, D], [1, n]])

    with tile.TileContext(nc) as tc:
        with ExitStack() as top:
            const = top.enter_context(tc.tile_pool(name="const", bufs=1))
            acts = top.enter_context(tc.tile_pool(name="acts", bufs=1))
            pqw = top.enter_context(tc.tile_pool(name="pqw", bufs=1))
            pq1 = top.enter_context(tc.tile_pool(name="pq1", bufs=1))

            bq_sb = const.tile([D, HPC], F32)
            # 1024 tiny descriptors: keep off the scalar/sync queues
            nc.gpsimd.dma_start(
                out=bq_sb, in_=bq.ap().rearrange("(h d) -> d h", d=D)
            )
            ones_sb = const.tile([D, D], BF)
            nc.vector.memset(ones_sb, 1.0)

            # persistent activations
            kT = [acts.tile([D, S], BF, name=f"kT{g}") for g in range(GPC)]
            vh = [acts.tile([D, KV], BF, name=f"vh{t}") for t in range(NT)]
            qh = [acts.tile([D, S], BF, name=f"qh{h}") for h in range(HPC)]
            cx = [acts.tile([D, S], BF, name=f"cx{h}") for h in range(HPC)]

            # ---- A-K: kT[d,t] = Wk_g^T x_k^T, streamed against xk DMA ----
            with tc.tile_pool(name="px", bufs=1) as px:
                wk_sb = px.tile([D, NE, KV], BF)
                nc.scalar.dma_start(
                    out=wk_sb, in_=wk.ap().rearrange("(n p) c -> p n c", p=D)
                )
                wv_sb = px.tile([D, NE, KV], BF)
                with tc.tile_pool(name="psK", bufs=1, space="PSUM") as psK:
                    ps_k = [
                        psK.tile([D, S], F32, name=f"psk{g}") for g in range(GPC)
                    ]
                    for e in range(NE):
                        xk_t = px.tile(
                            [D, S], BF, name=f"xk{e}", tag="xfull", bufs=3
                        )
                        nc.sync.dma_start(
                            out=xk_t, in_=xk.ap()[e * D : (e + 1) * D, :]
                        )
                        for g in range(GPC):
                            for m in range(4):
                                nc.tensor.matmul(
                                    ps_k[g][:, m * 512 : (m + 1) * 512],
                                    wk_sb[:, e, g * D : (g + 1) * D],
                                    xk_t[:, m * 512 : (m + 1) * 512],
                                    start=(e == 0),
                                    stop=(e == NE - 1),
                                )
                            if e == NE - 1:
                                # drain this group's banks while the other
                                # group's last matmuls run
                                for jj in range(2):
                                    nc.vector.tensor_copy(
                                        out=kT[g][:, jj * QW : (jj + 1) * QW],
                                        in_=ps_k[g][:, jj * QW : (jj + 1) * QW],
                                    )

                # ---- B (sq0, sq1): qh^T[d, 0:1024] = Wq_h^T x_q^T ----
                wq_sb = []
                xq0_ch = []
                for e in range(NE):
                    w_ = pqw.tile([D, QC], BF, name=f"wq{e}")
                    nc.sync.dma_start(out=w_, in_=wq_r[:, e, :])
                    wq_sb.append(w_)
                    t_ = px.tile([D, QW], BF, name=f"xq0_{e}", tag="xq0", bufs=16)
                    nc.sync.dma_start(out=t_, in_=xq.ap()[e * D : (e + 1) * D, 0:QW])
                    xq0_ch.append(t_)
                nc.sync.dma_start(
                    out=wv_sb, in_=wv.ap().rearrange("(n p) c -> p n c", p=D)
                )
                xq1_ch = []
                for e in range(NE):
                    t_ = pq1.tile([D, QW], BF, name=f"xq1_{e}")
                    nc.sync.dma_start(out=t_, in_=xq.ap()[e * D : (e + 1) * D, QW:S])
                    xq1_ch.append(t_)
                with tc.tile_pool(name="psB", bufs=1, space="PSUM") as psB:
                    for sq in range(2):
                        ps_q = [
                            psB.tile(
                                [D, 512], F32, name=f"psq{sq}_{h}",
                                tag="psq", bufs=8,
                            )
                            for h in range(HPC)
                        ]
                        for e in range(NE):
                            for h in range(HPC):
                                nc.tensor.matmul(
                                    ps_q[h],
                                    wq_sb[e][:, h * D : (h + 1) * D],
                                    xq0_ch[e][:, sq * 512 : (sq + 1) * 512],
                                    start=(e == 0),
                                    stop=(e == NE - 1),
                                )
                                if e == NE - 1:
                                    nc.vector.tensor_scalar_add(
                                        out=qh[h][:, sq * 512 : (sq + 1) * 512],
                                        in0=ps_q[h],
                                        scalar1=bq_sb[:, h : h + 1],
                                    )

                # ---- A-V: vh[t,d] = x_v @ Wv (stationary xv^T chunks) ----
                with tc.tile_pool(name="psV", bufs=1, space="PSUM") as psV:
                    ps_v = [
                        psV.tile([D, 2 * KV], F32, name=f"psv{i}")
                        for i in range(NT // 2)
                    ]
                    for e in range(NE):
                        xv_t = px.tile([D, S], BF, name=f"xv{e}", tag="xfull", bufs=3)
                        nc.sync.dma_start(
                            out=xv_t, in_=xv.ap()[e * D : (e + 1) * D, :]
                        )
                        for t in range(NT):
                            # two t-chunks share one PSUM bank: one group
                            # per bank (start zero-marks the whole bank)
                            nc.tensor.matmul(
                                ps_v[t // 2][:, (t % 2) * KV : (t % 2 + 1) * KV],
                                xv_t[:, t * D : (t + 1) * D],
                                wv_sb[:, e, :],
                                start=(e == 0 and t % 2 == 0),
                                stop=(e == NE - 1 and t % 2 == 1),
                            )
                            if e == NE - 1 and t % 2 == 1:
                                for u in (t - 1, t):
                                    nc.vector.tensor_copy(
                                        out=vh[u],
                                        in_=ps_v[u // 2][
                                            :, (u % 2) * KV : (u % 2 + 1) * KV
                                        ],
                                    )

            # ---- C (attention) with filler, then D (output projection) ----
            with (
                tc.tile_pool(name="pd", bufs=1) as pd,
                tc.tile_pool(name="pc", bufs=1) as pc,
                tc.tile_pool(name="psC", bufs=1, space="PSUM") as psC,
            ):
                wo_sb = pd.tile([D, HPC, E], BF)
                nc.sync.dma_start(out=wo_sb, in_=wo_r)
                bo_rep = pd.tile([D, E], F32)
                nc.sync.dma_start(out=bo_rep, in_=bcast(bo, E))

                # Filler generators: each next() emits one PE matmul (the
                # chain's trailing DVE/DMA ops ride along with its last MM).
                def gen_b_fillers():
                    for h in range(HPC):
                        for half in range(2):
                            sq = 2 + half
                            fp = psC.tile(
                                [D, 512], F32, name=f"fb{h}_{half}",
                                tag="fill", bufs=2,
                            )
                            for e in range(NE):
                                nc.tensor.matmul(
                                    fp,
                                    wq_sb[e][:, h * D : (h + 1) * D],
                                    xq1_ch[e][:, half * 512 : (half + 1) * 512],
                                    start=(e == 0),
                                    stop=(e == NE - 1),
                                )
                                if e == NE - 1:
                                    nc.vector.tensor_scalar_add(
                                        out=qh[h][:, sq * 512 : (sq + 1) * 512],
                                        in0=fp,
                                        scalar1=bq_sb[:, h : h + 1],
                                    )
                                yield

                def gen_d_fillers(ss_range):
                    for ss in ss_range:
                        for eh in range(4):
                            fp = psC.tile(
                                [D, 512], F32, name=f"fd{ss}_{eh}",
                                tag="fill", bufs=2,
                            )
                            for hh in range(HPC):
                                nc.tensor.matmul(
                                    fp,
                                    cx[hh][:, ss * D : (ss + 1) * D],
                                    wo_sb[:, hh, eh * 512 : (eh + 1) * 512],
                                    start=(hh == 0),
                                    stop=(hh == HPC - 1),
                                )
                                if hh == HPC - 1:
                                    ot = pc.tile(
                                        [D, 512], F32, name=f"ot{ss}_{eh}",
                                        tag="ot", bufs=2,
                                    )
                                    nc.vector.tensor_add(
                                        out=ot,
                                        in0=fp,
                                        in1=bo_rep[:, eh * 512 : (eh + 1) * 512],
                                    )
                                    nc.sync.dma_start(
                                        out=out.ap()[
                                            ss * D : (ss + 1) * D,
                                            eh * 512 : (eh + 1) * 512,
                                        ],
                                        in_=ot,
                                    )
                                yield

                fillers = [gen_b_fillers()]

                def pump(n):
                    for _ in range(n):
                        while fillers:
                            try:
                                next(fillers[0])
                                break
                            except StopIteration:
                                fillers.pop(0)
                        else:
                            return

                units = [(q2, h) for q2 in range(QH) for h in range(HPC)]

                def emit_scores(q2, h, t, lst):
                    g = h // (HPC // GPC)
                    p = psC.tile(
                        [D, QW], F32, name=f"pss{q2}_{h}_{t}", tag="pss", bufs=2
                    )
                    for j in range(2):
                        nc.tensor.matmul(
                            p[:, j * 512 : (j + 1) * 512],
                            kT[g][:, t * D : (t + 1) * D],
                            qh[h][:, q2 * QW + j * 512 : q2 * QW + (j + 1) * 512],
                            start=True, stop=True,
                        )
                    lst.append(p)

                ps_s_next = []
                for ui, (q2, h) in enumerate(units):
                    if q2 == 1 and h == 0:
                        fillers.append(gen_d_fillers(range(0, NT // 2)))
                    g = h // (HPC // GPC)
                    ps_s = ps_s_next
                    ps_s_next = []
                    ex = []
                    acc = pc.tile([D, QW], BF, name=f"acc{ui}", tag="acc", bufs=2)
                    ps_ctx = psC.tile(
                        [D, QW], F32, name=f"psctx{ui}", tag="psctx", bufs=1
                    )
                    if ui == 0:
                        emit_scores(q2, h, 0, ps_s)
                    for t in range(NT):
                        if t + 1 < NT:
                            emit_scores(q2, h, t + 1, ps_s)
                        elif ui + 1 < len(units):
                            # lookahead: next unit's first scores, so the
                            # scalar engine never drains across units
                            nq2, nh = units[ui + 1]
                            emit_scores(nq2, nh, 0, ps_s_next)
                        x_ = pc.tile(
                            [D, QW], BF, name=f"ex{ui}_{t}", tag="ex", bufs=3
                        )
                        nc.scalar.activation(
                            out=x_, in_=ps_s[t], func=Exp, scale=SCALE
                        )
                        ex.append(x_)
                        if t == 1:
                            nc.vector.tensor_add(out=acc, in0=ex[0], in1=x_)
                        elif t >= 2:
                            nc.vector.tensor_add(out=acc, in0=acc, in1=x_)
                        pump(2)
                        for j in range(2):
                            nc.tensor.matmul(
                                ps_ctx[:, j * 512 : (j + 1) * 512],
                                vh[t][:, g * D : (g + 1) * D],
                                ex[t][:, j * 512 : (j + 1) * 512],
                                start=(t == 0),
                                stop=(t == NT - 1),
                            )
                    # epilogue: drain ps_ctx (unnormalized, bf16), row sums
                    # on the pss rotation, then normalize off-critical-path
                    cxu = pc.tile([D, QW], BF, name=f"cxu{ui}", tag="cxu", bufs=2)
                    nc.vector.tensor_copy(out=cxu, in_=ps_ctx)
                    pump(1)
                    rs = psC.tile([D, QW], F32, name=f"rs{ui}", tag="pss", bufs=2)
                    for j in range(2):
                        nc.tensor.matmul(
                            rs[:, j * 512 : (j + 1) * 512],
                            ones_sb,
                            acc[:, j * 512 : (j + 1) * 512],
                            start=True, stop=True,
                        )
                    pump(2)
                    rr = pc.tile([D, QW], F32, name=f"rr{ui}", tag="rr", bufs=1)
                    nc.vector.reciprocal_approx_fast(out=rr, in_=rs)
                    nc.vector.tensor_mul(
                        out=cx[h][:, q2 * QW : (q2 + 1) * QW], in0=cxu, in1=rr
                    )

                # drain leftover filler, then the rest of D
                pump(1 << 30)
                fillers.append(gen_d_fillers(range(NT // 2, NT)))
                pump(1 << 30)

    nc.compile()
    return nc


def _get_program():
    global _PROGRAM
    if _PROGRAM is None:
        _PROGRAM = _build()
    return _PROGRAM


def make_in_maps(q, k, v, Wq, bq, Wk, bk, Wv, bv, Wo, bo):
    f32 = lambda a: np.asarray(a, dtype=np.float32)
    q, k, v = f32(q), f32(k), f32(v)
    Wq, bq, Wk, bk, Wv, bv, Wo, bo = (
        f32(Wq), f32(bq), f32(Wk), f32(bk), f32(Wv), f32(bv), f32(Wo), f32(bo)
    )
    in_maps = []
    xT = {}
    for b in range(B):
        xT[b] = (
            np.ascontiguousarray(q[b].T).astype(BF16),
            np.ascontiguousarray(k[b].T).astype(BF16),
            np.ascontiguousarray(v[b].T).astype(BF16),
        )
    halves = []
    for half in range(2):
        Wo_half = Wo[half * QC : (half + 1) * QC, :]
        bv_half = bv[half * KV : (half + 1) * KV]
        bv_exp = np.concatenate(
            [bv_half[(j // 4) * D : (j // 4 + 1) * D] for j in range(HPC)]
        )
        bo_eff = (bo if half == 0 else np.zeros_like(bo)).astype(
            np.float64
        ) + bv_exp.astype(np.float64) @ Wo_half.astype(np.float64)
        halves.append(
            {
                "wq": np.ascontiguousarray(
                    Wq[:, half * QC : (half + 1) * QC]
                ).astype(BF16),
                "wk": np.ascontiguousarray(
                    Wk[:, half * KV : (half + 1) * KV]
                ).astype(BF16),
                "wv": np.ascontiguousarray(
                    Wv[:, half * KV : (half + 1) * KV]
                ).astype(BF16),
                "wo": np.ascontiguousarray(Wo_half).astype(BF16),
                "bq": np.ascontiguousarray(bq[half * QC : (half + 1) * QC]),
                "bo": bo_eff.astype(np.float32),
            }
        )
    for c in range(N_CORES):
        b, half = c // 2, c % 2
        xqT, xkT, xvT = xT[b]
        in_maps.append({"xq": xqT, "xk": xkT, "xv": xvT, **halves[half]})
    return in_maps


def combine_results(results):
    out = np.empty((B, S, E), np.float32)
    for b in range(B):
        out[b] = np.asarray(results[2 * b]["out_p"]) + np.asarray(
            results[2 * b + 1]["out_p"]
        )
    return out


def kernel(q, k, v, Wq, bq, Wk, bk, Wv, bv, Wo, bo):
    from concourse.bass_utils import run_bass_kernel_spmd

    nc = _get_program()
    in_maps = make_in_maps(q, k, v, Wq, bq, Wk, bk, Wv, bv, Wo, bo)
    res = run_bass_kernel_spmd(nc, in_maps, core_ids=list(range(N_CORES)))
    return combine_results(res.results)


# revision 15
# speedup vs baseline: 1.0179x; 1.0179x over previous
"""GQA kernel for Trainium2, 8 NeuronCores.

Problem: nn_GroupQueryAttention — B=4, S=2048, E=2048, 16 heads / 4 groups,
d_head=128.  out = softmax((x@Wq) (x@Wk)^T / sqrt(d)) (x@Wv) @ Wo + biases.

Sharding: core c -> (batch b = c//2, half = c%2).  Each core handles one
batch and 2 of the 4 KV groups (= 8 of the 16 heads): Wq columns / Wo rows
split by head, Wk/Wv columns split by group.  Each core produces a partial
output projection for its batch; the host sums the two halves.

v4: v3 (C-phase filler pumping) + idle killers found in the v3 trace:
  - one rotating x pool (tags, fine-grained slot deps) instead of scoped
    pools whose closes serialized the DMA stream behind compute;
  - bq's 1024-descriptor gather moved to the idle gpsimd queue so wk/wq
    transfers start immediately; sync-queue transfer order = need order:
    wk, xk, (wq[e], xq0[e]) interleaved, wv, xq1, xv, wo, bo, outs;
  - PSUM-draining DVE ops (kT copies, qh bias adds, vh copies) interleaved
    with their producer matmuls so pool transitions don't stall the PE;
  - C units emit the NEXT unit's first scores tile before their epilogue
    (the scalar engine then flows exp->exp across unit boundaries; v3 lost
    2.4us/unit there), and ps_ctx is drained by an unnormalized bf16 copy
    (cxu) with row-sum/reciprocal/normalize trailing off the critical path.
  - numerics identical to v2/v3 (bf16 matmuls, fp32 accum, bk dropped,
    bv+bo folded host-side, bq folded on-chip, exp without max-subtract).
"""

import sys

sys.path.insert(0, "/opt/trn_rl_repo")

import numpy as np
import ml_dtypes

BF16 = ml_dtypes.bfloat16

B, S, E = 4, 2048, 2048
D = 128            # head dim
HPC = 8            # heads per core
GPC = 2            # groups per core
QC = HPC * D       # 1024 Wq cols per core
KV = GPC * D       # 256 Wk/Wv cols per core
NE = E // D        # 16 contraction chunks
NT = S // D        # 16 t-chunks of 128
QH = 2             # q halves of 1024
QW = S // QH       # 1024
N_CORES = 8

_PROGRAM = None


def _build():
    from contextlib import ExitStack

    import concourse.bass as bass
    import concourse.mybir as mybir
    import concourse.tile as tile
    from concourse import bacc

    F32 = mybir.dt.float32
    BF = mybir.dt.bfloat16
    Exp = mybir.ActivationFunctionType.Exp
    SCALE = 1.0 / float(np.sqrt(D))

    nc = bacc.Bacc("TRN2", target_bir_lowering=False, debug=False)
    # weights arrive pre-rearranged host-side so every DMA is a contiguous
    # per-partition stream (the on-DMA "(n p) c -> p n c" gathers were
    # 512B-segment descriptor storms that gated the kernel start)
    xq = nc.dram_tensor("xq", [E, S], BF, kind="ExternalInput")
    xk = nc.dram_tensor("xk", [E, S], BF, kind="ExternalInput")
    xv = nc.dram_tensor("xv", [E, S], BF, kind="ExternalInput")
    wq = nc.dram_tensor("wq", [D, NE, QC], BF, kind="ExternalInput")
    wk = nc.dram_tensor("wk", [D, NE, KV], BF, kind="ExternalInput")
    wv = nc.dram_tensor("wv", [D, NE, KV], BF, kind="ExternalInput")
    wo = nc.dram_tensor("wo", [D, HPC, E], BF, kind="ExternalInput")
    bq = nc.dram_tensor("bq", [D, HPC], F32, kind="ExternalInput")
    bo = nc.dram_tensor("bo", [E], F32, kind="ExternalInput")
    out = nc.dram_tensor("out_p", [S, E], F32, kind="ExternalOutput")

    def bcast(dram, n):
        return bass.AP(tensor=dram.ap().tensor, offset=0, ap=[[0, D], [1, n]])

    with tile.TileContext(nc) as tc:
        with ExitStack() as top:
            const = top.enter_context(tc.tile_pool(name="const", bufs=1))
            acts = top.enter_context(tc.tile_pool(name="acts", bufs=1))
            pqw = top.enter_context(tc.tile_pool(name="pqw", bufs=1))
            pq1 = top.enter_context(tc.tile_pool(name="pq1", bufs=1))

            bq_sb = const.tile([D, HPC], F32)
            nc.scalar.dma_start(out=bq_sb, in_=bq.ap())
            ones_sb = const.tile([D, D], BF)
            nc.vector.memset(ones_sb, 1.0)

            # persistent activations
            kT = [acts.tile([D, S], BF, name=f"kT{g}") for g in range(GPC)]
            vh = [acts.tile([D, KV], BF, name=f"vh{t}") for t in range(NT)]
            qh = [acts.tile([D, S], BF, name=f"qh{h}") for h in range(HPC)]
            cx = [acts.tile([D, S], BF, name=f"cx{h}") for h in range(HPC)]

            # ---- A-K: kT[d,t] = Wk_g^T x_k^T, streamed against xk DMA ----
            with tc.tile_pool(name="px", bufs=1) as px:
                wk_sb = px.tile([D, NE, KV], BF)
                nc.scalar.dma_start(out=wk_sb, in_=wk.ap())
                wv_sb = px.tile([D, NE, KV], BF)
                nc.scalar.dma_start(out=wv_sb, in_=wv.ap())
                with tc.tile_pool(name="psK", bufs=1, space="PSUM") as psK:
                    ps_k = [
                        psK.tile([D, S], F32, name=f"psk{g}") for g in range(GPC)
                    ]
                    for e in range(NE):
                        xk_t = px.tile(
                            [D, S], BF, name=f"xk{e}", tag="xfull", bufs=3
                        )
                        nc.sync.dma_start(
                            out=xk_t, in_=xk.ap()[e * D : (e + 1) * D, :]
                        )
                        for g in range(GPC):
                            for m in range(4):
                                nc.tensor.matmul(
                                    ps_k[g][:, m * 512 : (m + 1) * 512],
                                    wk_sb[:, e, g * D : (g + 1) * D],
                                    xk_t[:, m * 512 : (m + 1) * 512],
                                    start=(e == 0),
                                    stop=(e == NE - 1),
                                )
                            if e == NE - 1:
                                # drain this group's banks while the other
                                # group's last matmuls run
                                for jj in range(2):
                                    nc.vector.tensor_copy(
                                        out=kT[g][:, jj * QW : (jj + 1) * QW],
                                        in_=ps_k[g][:, jj * QW : (jj + 1) * QW],
                                    )

                # ---- B (sq0, sq1): qh^T[d, 0:1024] = Wq_h^T x_q^T ----
                wq_sb = []
                xq0_ch = []
                for e in range(NE):
                    w_ = pqw.tile([D, QC], BF, name=f"wq{e}")
                    nc.sync.dma_start(out=w_, in_=wq.ap()[:, e, :])
                    wq_sb.append(w_)
                    t_ = px.tile([D, QW], BF, name=f"xq0_{e}", tag="xq0", bufs=16)
                    nc.sync.dma_start(out=t_, in_=xq.ap()[e * D : (e + 1) * D, 0:QW])
                    xq0_ch.append(t_)
                # xq1 only feeds C-phase filler chains; xv (A-V, sooner) is
                # DMA'd first inside the A-V loop, xq1 after it
                with tc.tile_pool(name="psB", bufs=1, space="PSUM") as psB:
                    # 4-head quarter chains (4 banks each): the first chain
                    # only needs group 0's A-K banks, which drain first
                    for sq in range(2):
                        for hg in range(2):
                            ps_q = [
                                psB.tile(
                                    [D, 512], F32, name=f"psq{sq}_{hg}_{i}",
                                    tag="psq", bufs=8,
                                )
                                for i in range(4)
                            ]
                            for e in range(NE):
                                for i in range(4):
                                    h = hg * 4 + i
                                    nc.tensor.matmul(
                                        ps_q[i],
                                        wq_sb[e][:, h * D : (h + 1) * D],
                                        xq0_ch[e][:, sq * 512 : (sq + 1) * 512],
                                        start=(e == 0),
                                        stop=(e == NE - 1),
                                    )
                                    if e == NE - 1:
                                        nc.vector.tensor_scalar_add(
                                            out=qh[h][
                                                :, sq * 512 : (sq + 1) * 512
                                            ],
                                            in0=ps_q[i],
                                            scalar1=bq_sb[:, h : h + 1],
                                        )

                # ---- A-V: vh[t,d] = x_v @ Wv (stationary xv^T chunks) ----
                with tc.tile_pool(name="psV", bufs=1, space="PSUM") as psV:
                    ps_v = [
                        psV.tile([D, 2 * KV], F32, name=f"psv{i}")
                        for i in range(NT // 2)
                    ]
                    for e in range(NE):
                        xv_t = px.tile([D, S], BF, name=f"xv{e}", tag="xfull", bufs=3)
                        nc.sync.dma_start(
                            out=xv_t, in_=xv.ap()[e * D : (e + 1) * D, :]
                        )
                        for t in range(NT):
                            # two t-chunks share one PSUM bank: one group
                            # per bank (start zero-marks the whole bank)
                            nc.tensor.matmul(
                                ps_v[t // 2][:, (t % 2) * KV : (t % 2 + 1) * KV],
                                xv_t[:, t * D : (t + 1) * D],
                                wv_sb[:, e, :],
                                start=(e == 0 and t % 2 == 0),
                                stop=(e == NE - 1 and t % 2 == 1),
                            )
                            if e == NE - 1 and t % 2 == 1:
                                for u in (t - 1, t):
                                    nc.vector.tensor_copy(
                                        out=vh[u],
                                        in_=ps_v[u // 2][
                                            :, (u % 2) * KV : (u % 2 + 1) * KV
                                        ],
                                    )
                xq1_ch = []
                for e in range(NE):
                    t_ = pq1.tile([D, QW], BF, name=f"xq1_{e}")
                    nc.sync.dma_start(out=t_, in_=xq.ap()[e * D : (e + 1) * D, QW:S])
                    xq1_ch.append(t_)

            # ---- C (attention) with filler, then D (output projection) ----
            with (
                tc.tile_pool(name="pd", bufs=1) as pd,
                tc.tile_pool(name="pc", bufs=1) as pc,
                tc.tile_pool(name="psC", bufs=1, space="PSUM") as psC,
            ):
                wo_sb = pd.tile([D, HPC, E], BF)
                nc.scalar.dma_start(out=wo_sb, in_=wo.ap())
                bo_rep = pd.tile([D, E], F32)
                nc.scalar.dma_start(out=bo_rep, in_=bcast(bo, E))

                # Filler generators: each next() emits one PE matmul (the
                # chain's trailing DVE/DMA ops ride along with its last MM).
                def gen_b_fillers():
                    for h in range(HPC):
                        for half in range(2):
                            sq = 2 + half
                            fp = psC.tile(
                                [D, 512], F32, name=f"fb{h}_{half}",
                                tag="fill", bufs=2,
                            )
                            for e in range(NE):
                                nc.tensor.matmul(
                                    fp,
                                    wq_sb[e][:, h * D : (h + 1) * D],
                                    xq1_ch[e][:, half * 512 : (half + 1) * 512],
                                    start=(e == 0),
                                    stop=(e == NE - 1),
                                )
                                if e == NE - 1:
                                    nc.vector.tensor_scalar_add(
                                        out=qh[h][:, sq * 512 : (sq + 1) * 512],
                                        in0=fp,
                                        scalar1=bq_sb[:, h : h + 1],
                                    )
                                yield

                def gen_d_fillers(ss_range):
                    for ss in ss_range:
                        for eh in range(4):
                            fp = psC.tile(
                                [D, 512], F32, name=f"fd{ss}_{eh}",
                                tag="fill", bufs=2,
                            )
                            for hh in range(HPC):
                                nc.tensor.matmul(
                                    fp,
                                    cx[hh][:, ss * D : (ss + 1) * D],
                                    wo_sb[:, hh, eh * 512 : (eh + 1) * 512],
                                    start=(hh == 0),
                                    stop=(hh == HPC - 1),
                                )
                                if hh == HPC - 1:
                                    ot = pc.tile(
                                        [D, 512], F32, name=f"ot{ss}_{eh}",
                                        tag="ot", bufs=2,
                                    )
                                    nc.vector.tensor_add(
                                        out=ot,
                                        in0=fp,
                                        in1=bo_rep[:, eh * 512 : (eh + 1) * 512],
                                    )
                                    nc.sync.dma_start(
                                        out=out.ap()[
                                            ss * D : (ss + 1) * D,
                                            eh * 512 : (eh + 1) * 512,
                                        ],
                                        in_=ot,
                                    )
                                yield

                fillers = [gen_b_fillers()]

                def pump(n):
                    for _ in range(n):
                        while fillers:
                            try:
                                next(fillers[0])
                                break
                            except StopIteration:
                                fillers.pop(0)
                        else:
                            return

                units = [(q2, h) for q2 in range(QH) for h in range(HPC)]

                def emit_scores(q2, h, t, lst):
                    g = h // (HPC // GPC)
                    p = psC.tile(
                        [D, QW], F32, name=f"pss{q2}_{h}_{t}", tag="pss", bufs=2
                    )
                    for j in range(2):
                        nc.tensor.matmul(
                            p[:, j * 512 : (j + 1) * 512],
                            kT[g][:, t * D : (t + 1) * D],
                            qh[h][:, q2 * QW + j * 512 : q2 * QW + (j + 1) * 512],
                            start=True, stop=True,
                        )
                    lst.append(p)

                ps_s_next = []
                for ui, (q2, h) in enumerate(units):
                    if q2 == 1 and h == 0:
                        fillers.append(gen_d_fillers(range(0, NT // 2)))
                    g = h // (HPC // GPC)
                    ps_s = ps_s_next
                    ps_s_next = []
                    ex = []
                    acc = pc.tile([D, QW], BF, name=f"acc{ui}", tag="acc", bufs=2)
                    ps_ctx = psC.tile(
                        [D, QW], F32, name=f"psctx{ui}", tag="psctx", bufs=1
                    )
                    if ui == 0:
                        emit_scores(q2, h, 0, ps_s)
                    for t in range(NT):
                        if t + 1 < NT:
                            emit_scores(q2, h, t + 1, ps_s)
                        elif ui + 1 < len(units):
                            # lookahead: next unit's first scores, so the
                            # scalar engine never drains across units
                            nq2, nh = units[ui + 1]
                            emit_scores(nq2, nh, 0, ps_s_next)
                        x_ = pc.tile(
                            [D, QW], BF, name=f"ex{ui}_{t}", tag="ex", bufs=3
                        )
                        nc.scalar.activation(
                            out=x_, in_=ps_s[t], func=Exp, scale=SCALE
                        )
                        ex.append(x_)
                        if t == 1:
                            nc.vector.tensor_add(out=acc, in0=ex[0], in1=x_)
                        elif t >= 2:
                            nc.vector.tensor_add(out=acc, in0=acc, in1=x_)
                        pump(2)
                        for j in range(2):
                            nc.tensor.matmul(
                                ps_ctx[:, j * 512 : (j + 1) * 512],
                                vh[t][:, g * D : (g + 1) * D],
                                ex[t][:, j * 512 : (j + 1) * 512],
                                start=(t == 0),
                                stop=(t == NT - 1),
                            )
                    # epilogue: drain ps_ctx (unnormalized, bf16), row sums
                    # on the pss rotation, then normalize off-critical-path
                    cxu = pc.tile([D, QW], BF, name=f"cxu{ui}", tag="cxu", bufs=2)
                    nc.vector.tensor_copy(out=cxu, in_=ps_ctx)
                    pump(1)
                    rs = psC.tile([D, QW], F32, name=f"rs{ui}", tag="pss", bufs=2)
                    for j in range(2):
                        nc.tensor.matmul(
                            rs[:, j * 512 : (j + 1) * 512],
                            ones_sb,
                            acc[:, j * 512 : (j + 1) * 512],
                            start=True, stop=True,
                        )
                    pump(2)
                    rr = pc.tile([D, QW], F32, name=f"rr{ui}", tag="rr", bufs=1)
                    nc.vector.reciprocal_approx_fast(out=rr, in_=rs)
                    nc.vector.tensor_mul(
                        out=cx[h][:, q2 * QW : (q2 + 1) * QW], in0=cxu, in1=rr
                    )

                # drain leftover filler, then the rest of D
                pump(1 << 30)
                fillers.append(gen_d_fillers(range(NT // 2, NT)))
                pump(1 << 30)

    nc.compile()
    return nc


def _get_program():
    global _PROGRAM
    if _PROGRAM is None:
        _PROGRAM = _build()
    return _PROGRAM


def make_in_maps(q, k, v, Wq, bq, Wk, bk, Wv, bv, Wo, bo):
    f32 = lambda a: np.asarray(a, dtype=np.float32)
    q, k, v = f32(q), f32(k), f32(v)
    Wq, bq, Wk, bk, Wv, bv, Wo, bo = (
        f32(Wq), f32(bq), f32(Wk), f32(bk), f32(Wv), f32(bv), f32(Wo), f32(bo)
    )
    in_maps = []
    xT = {}
    for b in range(B):
        xT[b] = (
            np.ascontiguousarray(q[b].T).astype(BF16),
            np.ascontiguousarray(k[b].T).astype(BF16),
            np.ascontiguousarray(v[b].T).astype(BF16),
        )
    halves = []
    for half in range(2):
        Wo_half = Wo[half * QC : (half + 1) * QC, :]
        bv_half = bv[half * KV : (half + 1) * KV]
        bv_exp = np.concatenate(
            [bv_half[(j // 4) * D : (j // 4 + 1) * D] for j in range(HPC)]
        )
        bo_eff = (bo if half == 0 else np.zeros_like(bo)).astype(
            np.float64
        ) + bv_exp.astype(np.float64) @ Wo_half.astype(np.float64)
        # pre-rearrange host-side: row index (n*128+p) -> partition p, so
        # every weight DMA streams contiguously per partition
        def part3(w, cols):  # [E, cols] -> [D, NE, cols]
            return np.ascontiguousarray(
                w.reshape(NE, D, cols).transpose(1, 0, 2)
            ).astype(BF16)

        Wq_h = Wq[:, half * QC : (half + 1) * QC]
        Wk_h = Wk[:, half * KV : (half + 1) * KV]
        Wv_h = Wv[:, half * KV : (half + 1) * KV]
        halves.append(
            {
                "wq": part3(Wq_h, QC),
                "wk": part3(Wk_h, KV),
                "wv": part3(Wv_h, KV),
                "wo": np.ascontiguousarray(
                    Wo_half.reshape(HPC, D, E).transpose(1, 0, 2)
                ).astype(BF16),
                "bq": np.ascontiguousarray(
                    bq[half * QC : (half + 1) * QC].reshape(HPC, D).T
                ),
                "bo": bo_eff.astype(np.float32),
            }
        )
    for c in range(N_CORES):
        b, half = c // 2, c % 2
        xqT, xkT, xvT = xT[b]
        in_maps.append({"xq": xqT, "xk": xkT, "xv": xvT, **halves[half]})
    return in_maps


def combine_results(results):
    out = np.empty((B, S, E), np.float32)
    for b in range(B):
        out[b] = np.asarray(results[2 * b]["out_p"]) + np.asarray(
            results[2 * b + 1]["out_p"]
        )
    return out


def kernel(q, k, v, Wq, bq, Wk, bk, Wv, bv, Wo, bo):
    from concourse.bass_utils import run_bass_kernel_spmd

    nc = _get_program()
    in_maps = make_in_maps(q, k, v, Wq, bq, Wk, bk, Wv, bv, Wo, bo)
    res = run_bass_kernel_spmd(nc, in_maps, core_ids=list(range(N_CORES)))
    return combine_results(res.results)


# revision 19
# speedup vs baseline: 1.0261x; 1.0081x over previous
"""GQA kernel for Trainium2, 8 NeuronCores.

Problem: nn_GroupQueryAttention — B=4, S=2048, E=2048, 16 heads / 4 groups,
d_head=128.  out = softmax((x@Wq) (x@Wk)^T / sqrt(d)) (x@Wv) @ Wo + biases.

Sharding: core c -> (batch b = c//2, half = c%2).  Each core handles one
batch and 2 of the 4 KV groups (= 8 of the 16 heads): Wq columns / Wo rows
split by head, Wk/Wv columns split by group.  Each core produces a partial
output projection for its batch; the host sums the two halves.

v4: v3 (C-phase filler pumping) + idle killers found in the v3 trace:
  - one rotating x pool (tags, fine-grained slot deps) instead of scoped
    pools whose closes serialized the DMA stream behind compute;
  - bq's 1024-descriptor gather moved to the idle gpsimd queue so wk/wq
    transfers start immediately; sync-queue transfer order = need order:
    wk, xk, (wq[e], xq0[e]) interleaved, wv, xq1, xv, wo, bo, outs;
  - PSUM-draining DVE ops (kT copies, qh bias adds, vh copies) interleaved
    with their producer matmuls so pool transitions don't stall the PE;
  - C units emit the NEXT unit's first scores tile before their epilogue
    (the scalar engine then flows exp->exp across unit boundaries; v3 lost
    2.4us/unit there), and ps_ctx is drained by an unnormalized bf16 copy
    (cxu) with row-sum/reciprocal/normalize trailing off the critical path.
  - numerics identical to v2/v3 (bf16 matmuls, fp32 accum, bk dropped,
    bv+bo folded host-side, bq folded on-chip, exp without max-subtract).
"""

import sys

sys.path.insert(0, "/opt/trn_rl_repo")

import numpy as np
import ml_dtypes

BF16 = ml_dtypes.bfloat16

B, S, E = 4, 2048, 2048
D = 128            # head dim
HPC = 8            # heads per core
GPC = 2            # groups per core
QC = HPC * D       # 1024 Wq cols per core
KV = GPC * D       # 256 Wk/Wv cols per core
NE = E // D        # 16 contraction chunks
NT = S // D        # 16 t-chunks of 128
QH = 2             # q halves of 1024
QW = S // QH       # 1024
N_CORES = 8

_PROGRAM = None


def _build():
    from contextlib import ExitStack

    import concourse.bass as bass
    import concourse.mybir as mybir
    import concourse.tile as tile
    from concourse import bacc

    F32 = mybir.dt.float32
    BF = mybir.dt.bfloat16
    Exp = mybir.ActivationFunctionType.Exp
    SCALE = 1.0 / float(np.sqrt(D))

    nc = bacc.Bacc("TRN2", target_bir_lowering=False, debug=False)
    # weights arrive pre-rearranged host-side so every DMA is a contiguous
    # per-partition stream (the on-DMA "(n p) c -> p n c" gathers were
    # 512B-segment descriptor storms that gated the kernel start)
    xq = nc.dram_tensor("xq", [E, S], BF, kind="ExternalInput")
    xk = nc.dram_tensor("xk", [E, S], BF, kind="ExternalInput")
    xv = nc.dram_tensor("xv", [E, S], BF, kind="ExternalInput")
    wq = nc.dram_tensor("wq", [D, NE, QC], BF, kind="ExternalInput")
    wk = nc.dram_tensor("wk", [D, NE, KV], BF, kind="ExternalInput")
    wv = nc.dram_tensor("wv", [D, NE, KV], BF, kind="ExternalInput")
    wo = nc.dram_tensor("wo", [D, HPC, E], BF, kind="ExternalInput")
    bq = nc.dram_tensor("bq", [D, HPC], F32, kind="ExternalInput")
    bo = nc.dram_tensor("bo", [E], F32, kind="ExternalInput")
    out = nc.dram_tensor("out_p", [S, E], F32, kind="ExternalOutput")

    def bcast(dram, n):
        return bass.AP(tensor=dram.ap().tensor, offset=0, ap=[[0, D], [1, n]])

    with tile.TileContext(nc) as tc:
        with ExitStack() as top:
            const = top.enter_context(tc.tile_pool(name="const", bufs=1))
            acts = top.enter_context(tc.tile_pool(name="acts", bufs=1))
            pqw = top.enter_context(tc.tile_pool(name="pqw", bufs=1))
            pq1 = top.enter_context(tc.tile_pool(name="pq1", bufs=1))

            bq_sb = const.tile([D, HPC], F32)
            ones_sb = const.tile([D, D], BF)
            nc.vector.memset(ones_sb, 1.0)

            # persistent activations
            kT = [acts.tile([D, S], BF, name=f"kT{g}") for g in range(GPC)]
            vh = [acts.tile([D, KV], BF, name=f"vh{t}") for t in range(NT)]
            qh = [acts.tile([D, S], BF, name=f"qh{h}") for h in range(HPC)]
            cx = [acts.tile([D, S], BF, name=f"cx{h}") for h in range(HPC)]

            # ---- A-K: kT[d,t] = Wk_g^T x_k^T, streamed against xk DMA ----
            with tc.tile_pool(name="px", bufs=1) as px:
                # per-e wk tiles: the first A-K matmul gates on a 64KB
                # transfer instead of the whole 1MB
                wk_sb = []
                for e in range(NE):
                    w_ = px.tile([D, KV], BF, name=f"wk{e}")
                    nc.scalar.dma_start(out=w_, in_=wk.ap()[:, e, :])
                    wk_sb.append(w_)
                wv_sb = px.tile([D, NE, KV], BF)
                nc.scalar.dma_start(out=wv_sb, in_=wv.ap())
                nc.scalar.dma_start(out=bq_sb, in_=bq.ap())
                with tc.tile_pool(name="psK", bufs=1, space="PSUM") as psK:
                    ps_k = [
                        psK.tile([D, S], F32, name=f"psk{g}") for g in range(GPC)
                    ]
                    for e in range(NE):
                        xk_t = px.tile(
                            [D, S], BF, name=f"xk{e}", tag="xfull", bufs=3
                        )
                        nc.sync.dma_start(
                            out=xk_t, in_=xk.ap()[e * D : (e + 1) * D, :]
                        )
                        for g in range(GPC):
                            for m in range(4):
                                nc.tensor.matmul(
                                    ps_k[g][:, m * 512 : (m + 1) * 512],
                                    wk_sb[e][:, g * D : (g + 1) * D],
                                    xk_t[:, m * 512 : (m + 1) * 512],
                                    start=(e == 0),
                                    stop=(e == NE - 1),
                                )
                            if e == NE - 1:
                                # drain this group's banks while the other
                                # group's last matmuls run
                                for jj in range(2):
                                    nc.vector.tensor_copy(
                                        out=kT[g][:, jj * QW : (jj + 1) * QW],
                                        in_=ps_k[g][:, jj * QW : (jj + 1) * QW],
                                    )

                # ---- B (sq0, sq1): qh^T[d, 0:1024] = Wq_h^T x_q^T ----
                wq_sb = []
                xq0_ch = []
                for e in range(NE):
                    w_ = pqw.tile([D, QC], BF, name=f"wq{e}")
                    nc.sync.dma_start(out=w_, in_=wq.ap()[:, e, :])
                    wq_sb.append(w_)
                    t_ = px.tile([D, QW], BF, name=f"xq0_{e}", tag="xq0", bufs=16)
                    nc.sync.dma_start(out=t_, in_=xq.ap()[e * D : (e + 1) * D, 0:QW])
                    xq0_ch.append(t_)
                # xq1 only feeds C-phase filler chains; xv (A-V, sooner) is
                # DMA'd first inside the A-V loop, xq1 after it
                with tc.tile_pool(name="psB", bufs=1, space="PSUM") as psB:
                    # 8-head chains per s-quarter: 8 matmuls per xq0 chunk
                    # keeps PE consumption >= DMA arrival rate
                    for sq in range(2):
                        ps_q = [
                            psB.tile(
                                [D, 512], F32, name=f"psq{sq}_{h}",
                                tag="psq", bufs=8,
                            )
                            for h in range(HPC)
                        ]
                        for e in range(NE):
                            for h in range(HPC):
                                nc.tensor.matmul(
                                    ps_q[h],
                                    wq_sb[e][:, h * D : (h + 1) * D],
                                    xq0_ch[e][:, sq * 512 : (sq + 1) * 512],
                                    start=(e == 0),
                                    stop=(e == NE - 1),
                                )
                                if e == NE - 1:
                                    nc.vector.tensor_scalar_add(
                                        out=qh[h][:, sq * 512 : (sq + 1) * 512],
                                        in0=ps_q[h],
                                        scalar1=bq_sb[:, h : h + 1],
                                    )

                # ---- A-V: vh[t,d] = x_v @ Wv (stationary xv^T chunks) ----
                with tc.tile_pool(name="psV", bufs=1, space="PSUM") as psV:
                    ps_v = [
                        psV.tile([D, 2 * KV], F32, name=f"psv{i}")
                        for i in range(NT // 2)
                    ]
                    for e in range(NE):
                        xv_t = px.tile([D, S], BF, name=f"xv{e}", tag="xfull", bufs=3)
                        nc.sync.dma_start(
                            out=xv_t, in_=xv.ap()[e * D : (e + 1) * D, :]
                        )
                        for t in range(NT):
                            # two t-chunks share one PSUM bank: one group
                            # per bank (start zero-marks the whole bank)
                            nc.tensor.matmul(
                                ps_v[t // 2][:, (t % 2) * KV : (t % 2 + 1) * KV],
                                xv_t[:, t * D : (t + 1) * D],
                                wv_sb[:, e, :],
                                start=(e == 0 and t % 2 == 0),
                                stop=(e == NE - 1 and t % 2 == 1),
                            )
                            if e == NE - 1 and t % 2 == 1:
                                for u in (t - 1, t):
                                    nc.vector.tensor_copy(
                                        out=vh[u],
                                        in_=ps_v[u // 2][
                                            :, (u % 2) * KV : (u % 2 + 1) * KV
                                        ],
                                    )
                xq1_ch = []
                for e in range(NE):
                    t_ = pq1.tile([D, QW], BF, name=f"xq1_{e}")
                    nc.sync.dma_start(out=t_, in_=xq.ap()[e * D : (e + 1) * D, QW:S])
                    xq1_ch.append(t_)

            # ---- C (attention) with filler, then D (output projection) ----
            with (
                tc.tile_pool(name="pd", bufs=1) as pd,
                tc.tile_pool(name="pc", bufs=1) as pc,
                tc.tile_pool(name="psC", bufs=1, space="PSUM") as psC,
            ):
                wo_sb = pd.tile([D, HPC, E], BF)
                nc.scalar.dma_start(out=wo_sb, in_=wo.ap())
                bo_rep = pd.tile([D, E], F32)
                nc.scalar.dma_start(out=bo_rep, in_=bcast(bo, E))

                # Filler generators: each next() emits one PE matmul (the
                # chain's trailing DVE/DMA ops ride along with its last MM).
                def gen_b_fillers():
                    for h in range(HPC):
                        for half in range(2):
                            sq = 2 + half
                            fp = psC.tile(
                                [D, 512], F32, name=f"fb{h}_{half}",
                                tag="fill", bufs=2,
                            )
                            for e in range(NE):
                                nc.tensor.matmul(
                                    fp,
                                    wq_sb[e][:, h * D : (h + 1) * D],
                                    xq1_ch[e][:, half * 512 : (half + 1) * 512],
                                    start=(e == 0),
                                    stop=(e == NE - 1),
                                )
                                if e == NE - 1:
                                    nc.vector.tensor_scalar_add(
                                        out=qh[h][:, sq * 512 : (sq + 1) * 512],
                                        in0=fp,
                                        scalar1=bq_sb[:, h : h + 1],
                                    )
                                yield

                def gen_d_fillers(ss_range):
                    for ss in ss_range:
                        for eh in range(4):
                            fp = psC.tile(
                                [D, 512], F32, name=f"fd{ss}_{eh}",
                                tag="fill", bufs=2,
                            )
                            for hh in range(HPC):
                                nc.tensor.matmul(
                                    fp,
                                    cx[hh][:, ss * D : (ss + 1) * D],
                                    wo_sb[:, hh, eh * 512 : (eh + 1) * 512],
                                    start=(hh == 0),
                                    stop=(hh == HPC - 1),
                                )
                                if hh == HPC - 1:
                                    ot = pc.tile(
                                        [D, 512], F32, name=f"ot{ss}_{eh}",
                                        tag="ot", bufs=2,
                                    )
                                    nc.vector.tensor_add(
                                        out=ot,
                                        in0=fp,
                                        in1=bo_rep[:, eh * 512 : (eh + 1) * 512],
                                    )
                                    nc.sync.dma_start(
                                        out=out.ap()[
                                            ss * D : (ss + 1) * D,
                                            eh * 512 : (eh + 1) * 512,
                                        ],
                                        in_=ot,
                                    )
                                yield

                fillers = [gen_b_fillers()]

                def pump(n):
                    for _ in range(n):
                        while fillers:
                            try:
                                next(fillers[0])
                                break
                            except StopIteration:
                                fillers.pop(0)
                        else:
                            return

                units = [(q2, h) for q2 in range(QH) for h in range(HPC)]

                def emit_scores(q2, h, t, lst):
                    g = h // (HPC // GPC)
                    p = psC.tile(
                        [D, QW], F32, name=f"pss{q2}_{h}_{t}", tag="pss", bufs=2
                    )
                    for j in range(2):
                        nc.tensor.matmul(
                            p[:, j * 512 : (j + 1) * 512],
                            kT[g][:, t * D : (t + 1) * D],
                            qh[h][:, q2 * QW + j * 512 : q2 * QW + (j + 1) * 512],
                            start=True, stop=True,
                        )
                    lst.append(p)

                ps_s_next = []
                for ui, (q2, h) in enumerate(units):
                    if q2 == 1 and h == 0:
                        fillers.append(gen_d_fillers(range(0, NT // 2)))
                    g = h // (HPC // GPC)
                    ps_s = ps_s_next
                    ps_s_next = []
                    ex = []
                    acc = pc.tile([D, QW], BF, name=f"acc{ui}", tag="acc", bufs=2)
                    ps_ctx = psC.tile(
                        [D, QW], F32, name=f"psctx{ui}", tag="psctx", bufs=1
                    )
                    if ui == 0:
                        emit_scores(q2, h, 0, ps_s)
                    for t in range(NT):
                        if t + 1 < NT:
                            emit_scores(q2, h, t + 1, ps_s)
                        elif ui + 1 < len(units):
                            # lookahead: next unit's first scores, so the
                            # scalar engine never drains across units
                            nq2, nh = units[ui + 1]
                            emit_scores(nq2, nh, 0, ps_s_next)
                        x_ = pc.tile(
                            [D, QW], BF, name=f"ex{ui}_{t}", tag="ex", bufs=3
                        )
                        nc.scalar.activation(
                            out=x_, in_=ps_s[t], func=Exp, scale=SCALE
                        )
                        ex.append(x_)
                        if t == 1:
                            nc.vector.tensor_add(out=acc, in0=ex[0], in1=x_)
                        elif t >= 2:
                            nc.vector.tensor_add(out=acc, in0=acc, in1=x_)
                        pump(2)
                        for j in range(2):
                            nc.tensor.matmul(
                                ps_ctx[:, j * 512 : (j + 1) * 512],
                                vh[t][:, g * D : (g + 1) * D],
                                ex[t][:, j * 512 : (j + 1) * 512],
                                start=(t == 0),
                                stop=(t == NT - 1),
                            )
                    # epilogue: drain ps_ctx (unnormalized, bf16), row sums
                    # on the pss rotation, then normalize off-critical-path
                    cxu = pc.tile([D, QW], BF, name=f"cxu{ui}", tag="cxu", bufs=2)
                    nc.vector.tensor_copy(out=cxu, in_=ps_ctx)
                    pump(1)
                    rs = psC.tile([D, QW], F32, name=f"rs{ui}", tag="pss", bufs=2)
                    for j in range(2):
                        nc.tensor.matmul(
                            rs[:, j * 512 : (j + 1) * 512],
                            ones_sb,
                            acc[:, j * 512 : (j + 1) * 512],
                            start=True, stop=True,
                        )
                    pump(2)
                    rr = pc.tile([D, QW], F32, name=f"rr{ui}", tag="rr", bufs=1)
                    nc.vector.reciprocal_approx_fast(out=rr, in_=rs)
                    nc.vector.tensor_mul(
                        out=cx[h][:, q2 * QW : (q2 + 1) * QW], in0=cxu, in1=rr
                    )

                # drain leftover filler, then the rest of D
                pump(1 << 30)
                fillers.append(gen_d_fillers(range(NT // 2, NT)))
                pump(1 << 30)

    nc.compile()
    return nc


def _get_program():
    global _PROGRAM
    if _PROGRAM is None:
        _PROGRAM = _build()
    return _PROGRAM


def make_in_maps(q, k, v, Wq, bq, Wk, bk, Wv, bv, Wo, bo):
    f32 = lambda a: np.asarray(a, dtype=np.float32)
    q, k, v = f32(q), f32(k), f32(v)
    Wq, bq, Wk, bk, Wv, bv, Wo, bo = (
        f32(Wq), f32(bq), f32(Wk), f32(bk), f32(Wv), f32(bv), f32(Wo), f32(bo)
    )
    in_maps = []
    xT = {}
    for b in range(B):
        xT[b] = (
            np.ascontiguousarray(q[b].T).astype(BF16),
            np.ascontiguousarray(k[b].T).astype(BF16),
            np.ascontiguousarray(v[b].T).astype(BF16),
        )
    halves = []
    for half in range(2):
        Wo_half = Wo[half * QC : (half + 1) * QC, :]
        bv_half = bv[half * KV : (half + 1) * KV]
        bv_exp = np.concatenate(
            [bv_half[(j // 4) * D : (j // 4 + 1) * D] for j in range(HPC)]
        )
        bo_eff = (bo if half == 0 else np.zeros_like(bo)).astype(
            np.float64
        ) + bv_exp.astype(np.float64) @ Wo_half.astype(np.float64)
        # pre-rearrange host-side: row index (n*128+p) -> partition p, so
        # every weight DMA streams contiguously per partition
        def part3(w, cols):  # [E, cols] -> [D, NE, cols]
            return np.ascontiguousarray(
                w.reshape(NE, D, cols).transpose(1, 0, 2)
            ).astype(BF16)

        Wq_h = Wq[:, half * QC : (half + 1) * QC]
        Wk_h = Wk[:, half * KV : (half + 1) * KV]
        Wv_h = Wv[:, half * KV : (half + 1) * KV]
        halves.append(
            {
                "wq": part3(Wq_h, QC),
                "wk": part3(Wk_h, KV),
                "wv": part3(Wv_h, KV),
                "wo": np.ascontiguousarray(
                    Wo_half.reshape(HPC, D, E).transpose(1, 0, 2)
                ).astype(BF16),
                "bq": np.ascontiguousarray(
                    bq[half * QC : (half + 1) * QC].reshape(HPC, D).T
                ),
                "bo": bo_eff.astype(np.float32),
            }
        )
    for c in range(N_CORES):
        b, half = c // 2, c % 2
        xqT, xkT, xvT = xT[b]
        in_maps.append({"xq": xqT, "xk": xkT, "xv": xvT, **halves[half]})
    return in_maps


def combine_results(results):
    out = np.empty((B, S, E), np.float32)
    for b in range(B):
        out[b] = np.asarray(results[2 * b]["out_p"]) + np.asarray(
            results[2 * b + 1]["out_p"]
        )
    return out


def kernel(q, k, v, Wq, bq, Wk, bk, Wv, bv, Wo, bo):
    from concourse.bass_utils import run_bass_kernel_spmd

    nc = _get_program()
    in_maps = make_in_maps(q, k, v, Wq, bq, Wk, bk, Wv, bv, Wo, bo)
    res = run_bass_kernel_spmd(nc, in_maps, core_ids=list(range(N_CORES)))
    return combine_results(res.results)
